# revision 16
# baseline (speedup 1.0000x reference)
"""Trainium2 Bass kernel for the DeltaSynapse message-passing einsum.

Computes  I[b,o] = einsum('eo,dbe,deo,dbe->bo', signs*W, Xd, delaymap, Wshort)
with D=8, B=16, E=4096, O=4096, fp32.

Strategy (tensor-parallel over the post dim o, 8 cores, no collectives):
  - Each core owns a 512-wide o-shard of W, signs, delaymap and the output.
  - Xd/Wshort are tiny; they are transposed on the host to A_T layout
    [E, D*B] so the contraction dim e lands on SBUF partitions, and the
    product A = Xd*Wshort is formed on-chip.
  - Per e-chunk of 1024 rows: Weff = signs*W (DVE), then for each delay d,
    stream the 2 MB delaymap block, multiply by Weff in-place (DVE), and
    feed the TensorE with 8 accumulating matmuls (contract e, out [16,512]
    in a single PSUM bank).
All heavy traffic (64 MB/core of delaymap + 16 MB W/signs) streams as
contiguous 2 MB DMAs; the kernel is HBM-bandwidth bound.
"""

import sys

import numpy as np

sys.path.insert(0, "/opt/trn_rl_repo")

D, B, E, O = 8, 16, 4096, 4096
NCORES = 8
OS = O // NCORES        # 512: per-core o width
CH = 1024               # e-rows per DMA chunk (2 MB delaymap blocks)
NCH = E // CH           # 4 chunks
RP = CH // 128          # 8 e-rows per SBUF partition
DB = D * B              # 128

_CACHE = {}


def build_nc(mm_dtype_name="float32"):
    import concourse.mybir as mybir
    from concourse import bacc
    from concourse.tile import TileContext

    f32 = mybir.dt.float32
    mm_dt = getattr(mybir.dt, mm_dtype_name)

    # Bacc (not raw Bass): its compile pass splits multi-sem sync waits
    # into InstEventSemaphore, which the TT/TR ISA wait-slot limits require.
    nc = bacc.Bacc()
    dm = nc.dram_tensor("dm", [D, E, OS], f32, kind="ExternalInput")
    w = nc.dram_tensor("w", [E, OS], f32, kind="ExternalInput")
    sg = nc.dram_tensor("sg", [E, OS], f32, kind="ExternalInput")
    atx = nc.dram_tensor("atx", [E, DB], f32, kind="ExternalInput")
    atw = nc.dram_tensor("atw", [E, DB], f32, kind="ExternalInput")
    out = nc.dram_tensor("out", [B, OS], f32, kind="ExternalOutput")

    with TileContext(nc) as tc:
        with (
            tc.tile_pool(name="dmap", bufs=2) as dmap_pool,
            tc.tile_pool(name="mp", bufs=2) as m_pool,
            tc.tile_pool(name="wp", bufs=2) as w_pool,
            tc.tile_pool(name="sp", bufs=2) as s_pool,
            tc.tile_pool(name="atld", bufs=1) as atld_pool,
            tc.tile_pool(name="atp", bufs=1) as at_pool,
            tc.tile_pool(name="outp", bufs=1) as out_pool,
            tc.tile_pool(name="ps", bufs=1, space="PSUM") as psum_pool,
        ):
            # A_T = (Xd*Wshort) transposed to [e, d*B+b]; e on partitions.
            # Loaded chunk-by-chunk so the e->partition permutation matches
            # the delaymap/W tiles: within chunk c, e(p, j) = c*CH + RP*p + j.
            # The fp32r product tile is written ONLY by the DVE multiply
            # (the BIR verifier requires fp32r matmul operands to come from
            # instructions that round to fp32r).
            at_p = at_pool.tile([128, NCH * RP * DB], mm_dt, tag="atp")

            psum_t = psum_pool.tile([B, OS], f32)
            n_mm = NCH * D * RP
            mm = 0
            for c in range(NCH):
                cs = slice(c * RP * DB, (c + 1) * RP * DB)
                at_xt = atld_pool.tile([128, RP * DB], f32, tag="atx")
                at_wt = atld_pool.tile([128, RP * DB], f32, tag="atw")
                nc.sync.dma_start(
                    out=at_xt,
                    in_=atx[c * CH:(c + 1) * CH, :].rearrange(
                        "(p r) k -> p (r k)", p=128))
                nc.sync.dma_start(
                    out=at_wt,
                    in_=atw[c * CH:(c + 1) * CH, :].rearrange(
                        "(p r) k -> p (r k)", p=128))
                nc.vector.tensor_mul(at_p[:, cs], at_xt, at_wt)

                w_t = w_pool.tile([128, RP * OS], f32, tag="w")
                s_t = s_pool.tile([128, RP * OS], f32, tag="s")
                nc.sync.dma_start(
                    out=w_t,
                    in_=w[c * CH:(c + 1) * CH, :].rearrange(
                        "(p r) o -> p (r o)", p=128))
                sg_t = w_pool.tile([128, RP * OS], f32, tag="sg")
                nc.sync.dma_start(
                    out=sg_t,
                    in_=sg[c * CH:(c + 1) * CH, :].rearrange(
                        "(p r) o -> p (r o)", p=128))
                # Weff chunk: s_t <- signs * W  (fp32 inputs, mm_dt output)
                nc.vector.tensor_mul(s_t, sg_t, w_t)
                for d in range(D):
                    dm_t = dmap_pool.tile([128, RP * OS], f32, tag="dm")
                    # delaymap stream rides the second HWDGE ring (ACT) so
                    # it doesn't queue behind the W/signs/A_T loads on SP
                    nc.scalar.dma_start(
                        out=dm_t,
                        in_=dm[d, c * CH:(c + 1) * CH, :].rearrange(
                            "(p r) o -> p (r o)", p=128))
                    m_t = m_pool.tile([128, RP * OS], mm_dt, tag="m")
                    # m_t <- delaymap * Weff
                    nc.vector.tensor_mul(m_t, dm_t, s_t)
                    for j in range(RP):
                        lhsT = at_p[:, c * RP * DB + j * DB + d * B:
                                    c * RP * DB + j * DB + d * B + B]
                        rhs = m_t[:, j * OS:(j + 1) * OS]
                        nc.tensor.matmul(
                            psum_t, lhsT=lhsT, rhs=rhs,
                            start=(mm == 0), stop=(mm == n_mm - 1))
                        mm += 1

            out_t = out_pool.tile([B, OS], f32)
            nc.vector.tensor_copy(out_t, psum_t)
            nc.sync.dma_start(out=out[:, :], in_=out_t)

    # run Bacc's compile passes (incl. sync-wait splitting) now; the
    # bass2jax exec path binds the primitive without finalizing.
    nc.finalize()
    return nc


def _get_nc():
    if "nc" not in _CACHE:
        _CACHE["nc"] = build_nc()
    return _CACHE["nc"]


def prepare_in_maps(W, signs, Xd, delaymap, Wshort):
    W = np.asarray(W, dtype=np.float32)
    signs = np.asarray(signs, dtype=np.float32)
    Xd = np.asarray(Xd, dtype=np.float32)
    delaymap = np.asarray(delaymap, dtype=np.float32)
    Wshort = np.asarray(Wshort, dtype=np.float32)

    # host-side layout prep: [D,B,E] -> [E, D*B] so e is the partition dim
    atx = np.ascontiguousarray(Xd.transpose(2, 0, 1).reshape(E, DB))
    atw = np.ascontiguousarray(Wshort.transpose(2, 0, 1).reshape(E, DB))

    in_maps = []
    for m in range(NCORES):
        sl = slice(m * OS, (m + 1) * OS)
        in_maps.append({
            "dm": np.ascontiguousarray(delaymap[:, :, sl]),
            "w": np.ascontiguousarray(W[:, sl]),
            "sg": np.ascontiguousarray(signs[:, sl]),
            "atx": atx,
            "atw": atw,
        })
    return in_maps


def kernel(W, signs, Xd, delaymap, Wshort):
    from concourse.bass_utils import run_bass_kernel_spmd

    in_maps = prepare_in_maps(W, signs, Xd, delaymap, Wshort)
    nc = _get_nc()
    res = run_bass_kernel_spmd(nc, in_maps, core_ids=list(range(NCORES)))
    return np.concatenate([r["out"] for r in res.results], axis=1)
